# revision 11
# baseline (speedup 1.0000x reference)
"""IsoMax pairwise-distance kernel for 8 TRN2 NeuronCores.

Math:  out[b,m] = -|s| * sqrt(max(||xn_b||^2 + ||pn_m||^2 - 2*xn_b.pn_m, 0))
with xn/pn L2-normalized rows of x [4096,2048] and prototypes [12893,2048].
Since xn,pn are unit vectors this is -|s|*sqrt(2 - 2*cos).

fp8 path: G' = fp8(x) @ fp8(-16*pn)^T via DoubleRow perf mode (2 contraction
rows per PE cycle), accumulated f32 in PSUM. pn is NEGATED in the prologue so
the epilogue scale svec = s^2/(8*||x_b||) is positive and needs only a DVE
reciprocal (no extra sign flip): sqrt(svec*G' + 2s^2) = s*sqrt(2-2cos).
The device returns +|s|*dist in bf16; the host negates during the f32 upcast.

Sharding: prototypes split across the 8 cores (output columns), x replicated.
M=12893 padded to 13312 = 8*1664 (zero rows -> harmless, sliced off on host).

Dataflow (steady state): the only matmul-gated ops are the epilogue and the
store; everything upstream (loads, transposes, casts, norms) is pipelined
ahead so the PE never waits and its p-state ramps to 2.4 GHz.
  SWDGE:  x f32->bf16 loads, XPIPE tiles ahead
  Sync:   xT/pT transposes (TPIPE ahead) + output stores
  DVE:    sum(x^2) (tensor_tensor_reduce), fp8 casts (1 ahead), reciprocal
  ACT:    norm sqrt + 4-chunk sqrt epilogue (emitted sq-before-epi so the
          PSUM banks free as early as possible)
  PE:     32 DoubleRow matmuls per tile (4 psum chunks x 8 k-pairs)
"""

import os
import sys

sys.path.insert(0, "/opt/trn_rl_repo")

import numpy as np

B = 4096
D = 2048
M_FULL = 12893
N_CORES = 8
MC = 1664  # per-core prototype rows (13*128); 8*1664 = 13312 >= 12893
P = 128
KT = D // P  # 16 contraction chunks
MT = MC // P  # 13 m-tiles per core
BT = B // P  # 32 b-tiles

SCALE_P = 16.0  # fp8 range scaling for normalized prototypes

_cache = {}


def _build(s_abs: float, b_rows: int = B, mc: int = MC):
    import concourse.bass as bass  # noqa: F401
    import concourse.mybir as mybir
    import concourse.tile as tile
    from concourse import bacc
    from contextlib import ExitStack

    f32 = mybir.dt.float32
    bf16 = mybir.dt.bfloat16
    fp8 = mybir.dt.float8e4
    AF = mybir.ActivationFunctionType
    PM = mybir.MatmulPerfMode
    ALU = mybir.AluOpType
    kt = D // P
    mt_n = mc // P
    bt_n = b_rows // P
    two_s2 = 2.0 * s_abs * s_abs
    # sqrt(norm_scale * ||x||^2) = 8*||x||/s^2 ; its reciprocal is the
    # (positive) epilogue scale s^2/(8*||x||) = 2s^2/(16*||x||)
    norm_scale = 64.0 / (s_abs**4)

    # psum chunks over mc columns (<=512 wide, multiples of 128)
    chunks = []
    off = 0
    while off < mc:
        w = min(512, mc - off)
        chunks.append((off, w))
        off += w

    XPIPE = 4  # x-load prefetch depth (bounded by xpool bufs)
    TPIPE = 2  # transpose lookahead (bounded by xtpool bufs)

    nc = bacc.Bacc(None, target_bir_lowering=False)
    x_d = nc.dram_tensor("x", [b_rows, D], f32, kind="ExternalInput")
    p_d = nc.dram_tensor("p", [mc, D], f32, kind="ExternalInput")
    o_d = nc.dram_tensor("o", [b_rows, mc], bf16, kind="ExternalOutput")

    with ExitStack() as ctx:
        tc = ctx.enter_context(tile.TileContext(nc))
        persist = ctx.enter_context(tc.tile_pool(name="persist", bufs=1))
        # all p loads are emitted upfront on Sync; bufs >= mt_n so none of
        # them waits on a downstream consumer (which would stall the queue)
        ppool = ctx.enter_context(tc.tile_pool(name="ppool", bufs=min(mt_n, 13)))
        ptmp = ctx.enter_context(tc.tile_pool(name="ptmp", bufs=2))
        sq = ctx.enter_context(tc.tile_pool(name="sq", bufs=2))
        small = ctx.enter_context(tc.tile_pool(name="small", bufs=8))
        xpool = ctx.enter_context(tc.tile_pool(name="xpool", bufs=4))
        xtpool = ctx.enter_context(tc.tile_pool(name="xtpool", bufs=3))
        x8pool = ctx.enter_context(tc.tile_pool(name="x8pool", bufs=3))
        opool = ctx.enter_context(tc.tile_pool(name="opool", bufs=3))
        psum = ctx.enter_context(tc.tile_pool(name="psum", bufs=8, space="PSUM"))

        # prototypes, -16/||p|| scaled, fp8, transposed: [d_inner, k, m]
        pT8 = persist.tile([P, kt, mc], fp8)

        # bias tiles for ACT (const-AP db has no arbitrary constants)
        tiny_b = persist.tile([P, 1], f32, tag="tiny_b")
        nc.vector.memset(tiny_b, 1e-30)
        two_s2_b = persist.tile([P, 1], f32, tag="two_s2_b")
        nc.vector.memset(two_s2_b, two_s2)

        # ---- x-load prefetch (SWDGE is free during the prologue) ----
        x_bfs = {}

        def load_x(i):
            if i >= bt_n:
                return
            t = xpool.tile([P, D], bf16, tag="x_bf")
            # SWDGE dma casts f32 -> bf16 in flight
            nc.gpsimd.dma_start(t, x_d[i * P : (i + 1) * P, :])
            x_bfs[i] = t

        xT_bfs = {}

        def trans_x(i):
            if i >= bt_n:
                return
            t = xtpool.tile([P, kt, P], bf16, tag="xT")
            nc.sync.dma_start(t, x_bfs[i], transpose=True)
            xT_bfs[i] = t

        xT8s = {}

        def cast_x(i):
            if i >= bt_n:
                return
            t = x8pool.tile([P, kt, P], fp8, tag="xT8")
            nc.vector.tensor_scalar_mul(t, xT_bfs.pop(i), 1.0)
            xT8s[i] = t

        svecs = {}

        def norms(i):
            """ssx = sum x^2 (DVE), xnorm_s = sqrt(64/s^4 * ssx) (ACT),
            svec = 1/xnorm_s (DVE) = s^2/(8||x||), positive."""
            if i >= bt_n:
                return
            xsq = sq.tile([P, D], bf16, tag="sq")
            ssx = small.tile([P, 1], f32, tag="ss")
            nc.scalar.activation(xsq, x_bfs[i], AF.Square, accum_out=ssx)
            xnorm = small.tile([P, 1], f32, tag="nrm")
            nc.scalar.activation(xnorm, ssx, AF.Sqrt, bias=tiny_b)
            rx = small.tile([P, 1], f32, tag="rx")
            nc.vector.reciprocal(rx, xnorm)
            svec = small.tile([P, 1], f32, tag="svec")
            nc.vector.tensor_scalar_mul(svec, rx, -two_s2 / SCALE_P)
            svecs[i] = svec

        for i in range(min(XPIPE, bt_n)):
            load_x(i)

        # ---- prologue: all p loads fire upfront on Sync, then per-m-tile
        # normalize (negated) + transpose + fp8 cast ----
        p_tiles = []
        for mt in range(mt_n):
            p_f = ppool.tile([P, D], f32, tag="p_f")
            nc.sync.dma_start(p_f, p_d[mt * P : (mt + 1) * P, :])
            p_tiles.append(p_f)
        for mt in range(mt_n):
            p_f = p_tiles[mt]
            psq = sq.tile([P, D], bf16, tag="sq")
            ssp = small.tile([P, 1], f32, tag="ss")
            # ssp = sum_d p^2 (Square+Sqrt share one ACT table set)
            nc.scalar.activation(psq, p_f, AF.Square, accum_out=ssp)
            pnorm = small.tile([P, 1], f32, tag="nrm")
            nc.scalar.activation(pnorm, ssp, AF.Sqrt, bias=tiny_b)
            rp = small.tile([P, 1], f32, tag="rp")
            nc.vector.reciprocal(rp, pnorm)
            rps = small.tile([P, 1], f32, tag="rps")
            nc.vector.tensor_scalar_mul(rps, rp, SCALE_P)
            pn = ptmp.tile([P, D], bf16, tag="pn")
            nc.vector.tensor_scalar_mul(pn, p_f, rps)
            pT_bf = ptmp.tile([P, kt, P], bf16, tag="pT_bf")
            nc.sync.dma_start(pT_bf, pn, transpose=True)
            # cast into the persistent fp8 slab (strided dst) on DVE
            nc.vector.tensor_scalar_mul(
                pT8[:, :, mt * P : (mt + 1) * P], pT_bf, 1.0
            )

        for i in range(min(TPIPE, bt_n)):
            trans_x(i)
        cast_x(0)
        norms(0)

        # ---- software-pipelined b-loop ----
        for bt in range(bt_n):
            load_x(bt + XPIPE)
            trans_x(bt + TPIPE)
            cast_x(bt + 1)
            norms(bt + 1)

            xT8 = xT8s.pop(bt)
            x_bfs.pop(bt)
            pts = [
                psum.tile([P, 512], f32, tag="ps", name=f"ps_{ci}")[:, :w]
                for ci, (_o, w) in enumerate(chunks)
            ]
            # chunk-major: finish one PSUM bank's accumulation group first so
            # the ACT epilogue starts while later chunks still matmul
            for ci, (coff, w) in enumerate(chunks):
                for j in range(kt // 2):
                    nc.tensor.matmul(
                        pts[ci],
                        xT8[:, 2 * j : 2 * j + 2, :],
                        pT8[:, 2 * j : 2 * j + 2, coff : coff + w],
                        start=(j == 0),
                        stop=(j == kt // 2 - 1),
                        perf_mode=PM.DoubleRow,
                    )
            t_sb = opool.tile([P, mc], bf16, tag="t_sb")
            svec = svecs.pop(bt)
            for ci, (coff, w) in enumerate(chunks):
                # sqrt(s^2/(8||x||) * G' + 2s^2) = s*sqrt(2 - 2*cos)
                nc.scalar.activation(
                    t_sb[:, coff : coff + w], pts[ci], AF.Sqrt,
                    bias=two_s2_b, scale=svec,
                )
            nc.sync.dma_start(o_d[bt * P : (bt + 1) * P, :], t_sb)

    nc.compile()
    return nc


LAST_RESULT = None


def _run(nc, in_maps, core_ids):
    from concourse import bass_utils

    global LAST_RESULT
    trace = bool(int(os.environ.get("ISOMAX_TRACE", "0")))
    LAST_RESULT = bass_utils.run_bass_kernel_spmd(
        nc, in_maps, core_ids=core_ids, trace=trace
    )
    return LAST_RESULT.results


def kernel(x, prototypes, distance_scale):
    x = np.ascontiguousarray(np.asarray(x, dtype=np.float32))
    p = np.asarray(prototypes, dtype=np.float32)
    s_abs = float(abs(np.asarray(distance_scale).reshape(-1)[0].item()))
    m, d = p.shape
    assert (m, d) == (M_FULL, D) and x.shape == (B, D)

    key = ("fp8", s_abs)
    if key not in _cache:
        _cache[key] = _build(s_abs)
    nc = _cache[key]

    p_pad = np.zeros((N_CORES * MC, D), np.float32)
    p_pad[:m] = p
    in_maps = [
        {"x": x, "p": np.ascontiguousarray(p_pad[i * MC : (i + 1) * MC])}
        for i in range(N_CORES)
    ]
    results = _run(nc, in_maps, list(range(N_CORES)))
    out = np.concatenate(
        [np.asarray(results[i]["o"]) for i in range(N_CORES)], axis=1
    )
    # device emits +|s|*dist; negate during the f32 upcast
    return -(out[:, :m].astype(np.float32))


# revision 12
# speedup vs baseline: 1.0716x; 1.0716x over previous
"""IsoMax pairwise-distance kernel for 8 TRN2 NeuronCores.

Math:  out[b,m] = -|s| * sqrt(max(||xn_b||^2 + ||pn_m||^2 - 2*xn_b.pn_m, 0))
with xn/pn L2-normalized rows of x [4096,2048] and prototypes [12893,2048].
Since xn,pn are unit vectors this is -|s|*sqrt(2 - 2*cos).

fp8 path: G' = fp8(x) @ fp8(-16*pn)^T via DoubleRow perf mode (2 contraction
rows per PE cycle), accumulated f32 in PSUM. pn is NEGATED in the prologue so
the epilogue scale svec = s^2/(8*||x_b||) is positive and needs only a DVE
reciprocal (no extra sign flip): sqrt(svec*G' + 2s^2) = s*sqrt(2-2cos).
The device returns +|s|*dist in bf16; the host negates during the f32 upcast.

Sharding: prototypes split across the 8 cores (output columns), x replicated.
M=12893 padded to 13312 = 8*1664 (zero rows -> harmless, sliced off on host).

Dataflow (steady state): the only matmul-gated ops are the epilogue and the
store; everything upstream (loads, transposes, casts, norms) is pipelined
ahead so the PE never waits and its p-state ramps to 2.4 GHz.
  SWDGE:  x f32->bf16 loads, XPIPE tiles ahead
  Sync:   xT/pT transposes (TPIPE ahead) + output stores
  DVE:    sum(x^2) (tensor_tensor_reduce), fp8 casts (1 ahead), reciprocal
  ACT:    norm sqrt + 4-chunk sqrt epilogue (emitted sq-before-epi so the
          PSUM banks free as early as possible)
  PE:     32 DoubleRow matmuls per tile (4 psum chunks x 8 k-pairs)
"""

import os
import sys

sys.path.insert(0, "/opt/trn_rl_repo")

import numpy as np

B = 4096
D = 2048
M_FULL = 12893
N_CORES = 8
MC = 1664  # per-core prototype rows (13*128); 8*1664 = 13312 >= 12893
P = 128
KT = D // P  # 16 contraction chunks
MT = MC // P  # 13 m-tiles per core
BT = B // P  # 32 b-tiles

SCALE_P = 16.0  # fp8 range scaling for normalized prototypes

_cache = {}


def _build(s_abs: float, b_rows: int = B, mc: int = MC):
    import concourse.bass as bass  # noqa: F401
    import concourse.mybir as mybir
    import concourse.tile as tile
    from concourse import bacc
    from contextlib import ExitStack

    f32 = mybir.dt.float32
    bf16 = mybir.dt.bfloat16
    fp8 = mybir.dt.float8e4
    AF = mybir.ActivationFunctionType
    PM = mybir.MatmulPerfMode
    ALU = mybir.AluOpType
    kt = D // P
    mt_n = mc // P
    bt_n = b_rows // P
    two_s2 = 2.0 * s_abs * s_abs
    # sqrt(norm_scale * ||x||^2) = 8*||x||/s^2 ; its reciprocal is the
    # (positive) epilogue scale s^2/(8*||x||) = 2s^2/(16*||x||)
    norm_scale = 64.0 / (s_abs**4)

    # psum chunks over mc columns (<=512 wide, multiples of 128)
    chunks = []
    off = 0
    while off < mc:
        w = min(512, mc - off)
        chunks.append((off, w))
        off += w

    XPIPE = 5  # x-load prefetch depth (bounded by xpool bufs)
    TPIPE = 4  # transpose lookahead: stores on Sync sit between transposes,
               # so the lookahead absorbs the store's wait-for-epilogue

    nc = bacc.Bacc(None, target_bir_lowering=False)
    x_d = nc.dram_tensor("x", [b_rows, D], f32, kind="ExternalInput")
    p_d = nc.dram_tensor("p", [mc, D], f32, kind="ExternalInput")
    o_d = nc.dram_tensor("o", [b_rows, mc], bf16, kind="ExternalOutput")

    with ExitStack() as ctx:
        tc = ctx.enter_context(tile.TileContext(nc))
        persist = ctx.enter_context(tc.tile_pool(name="persist", bufs=1))
        # wave-A p loads fill ppool upfront; wave-B loads are emitted as
        # triggers interleaved with the prologue chains (bufs freed by pn)
        ppool = ctx.enter_context(tc.tile_pool(name="ppool", bufs=min(mt_n, 7)))
        pnpool = ctx.enter_context(tc.tile_pool(name="pnpool", bufs=3))
        ptpool = ctx.enter_context(tc.tile_pool(name="ptpool", bufs=5))
        sq = ctx.enter_context(tc.tile_pool(name="sq", bufs=2))
        small = ctx.enter_context(tc.tile_pool(name="small", bufs=8))
        xpool = ctx.enter_context(tc.tile_pool(name="xpool", bufs=6))
        xtpool = ctx.enter_context(tc.tile_pool(name="xtpool", bufs=6))
        x8pool = ctx.enter_context(tc.tile_pool(name="x8pool", bufs=3))
        opool = ctx.enter_context(tc.tile_pool(name="opool", bufs=3))
        psum = ctx.enter_context(tc.tile_pool(name="psum", bufs=8, space="PSUM"))

        # prototypes, -16/||p|| scaled, fp8, transposed: [d_inner, k, m]
        pT8 = persist.tile([P, kt, mc], fp8)

        # bias tiles for ACT (const-AP db has no arbitrary constants)
        tiny_b = persist.tile([P, 1], f32, tag="tiny_b")
        nc.vector.memset(tiny_b, 1e-30)
        two_s2_b = persist.tile([P, 1], f32, tag="two_s2_b")
        nc.vector.memset(two_s2_b, two_s2)

        # ---- x-load prefetch (SWDGE is free during the prologue) ----
        x_bfs = {}

        def load_x(i):
            if i >= bt_n:
                return
            t = xpool.tile([P, D], bf16, tag="x_bf")
            # SWDGE dma casts f32 -> bf16 in flight
            nc.gpsimd.dma_start(t, x_d[i * P : (i + 1) * P, :])
            x_bfs[i] = t

        xT_bfs = {}

        def trans_x(i):
            if i >= bt_n:
                return
            t = xtpool.tile([P, kt, P], bf16, tag="xT")
            nc.sync.dma_start(t, x_bfs[i], transpose=True)
            xT_bfs[i] = t

        xT8s = {}

        def cast_x(i):
            if i >= bt_n:
                return
            t = x8pool.tile([P, kt, P], fp8, tag="xT8")
            nc.vector.tensor_scalar_mul(t, xT_bfs.pop(i), 1.0)
            xT8s[i] = t

        svecs = {}

        def norms(i):
            """ssx = sum x^2 (DVE), xnorm_s = sqrt(64/s^4 * ssx) (ACT),
            svec = 1/xnorm_s (DVE) = s^2/(8||x||), positive."""
            if i >= bt_n:
                return
            xsq = sq.tile([P, D], bf16, tag="sq")
            ssx = small.tile([P, 1], f32, tag="ss")
            nc.scalar.activation(xsq, x_bfs[i], AF.Square, accum_out=ssx)
            xnorm = small.tile([P, 1], f32, tag="nrm")
            nc.scalar.activation(xnorm, ssx, AF.Sqrt, bias=tiny_b)
            rx = small.tile([P, 1], f32, tag="rx")
            nc.vector.reciprocal(rx, xnorm)
            svec = small.tile([P, 1], f32, tag="svec")
            nc.vector.tensor_scalar_mul(svec, rx, -two_s2 / SCALE_P)
            svecs[i] = svec

        for i in range(min(XPIPE, bt_n)):
            load_x(i)

        # ---- prologue: wave-A p loads upfront; per-m-tile normalize +
        # transpose; the DVE fp8 cast is delayed 3 m-tiles so it never
        # head-of-line blocks the next pn-mul waiting on its transpose ----
        wave_a = min(7, mt_n)
        p_tiles = {}
        for mt in range(wave_a):
            p_f = ppool.tile([P, D], f32, tag="p_f")
            nc.sync.dma_start(p_f, p_d[mt * P : (mt + 1) * P, :])
            p_tiles[mt] = p_f
        pT_bfs = {}

        def cast_p(mt):
            if not (0 <= mt < mt_n):
                return
            nc.vector.tensor_scalar_mul(
                pT8[:, :, mt * P : (mt + 1) * P], pT_bfs.pop(mt), 1.0
            )

        for mt in range(mt_n):
            p_f = p_tiles.pop(mt)
            psq = sq.tile([P, D], bf16, tag="sq")
            ssp = small.tile([P, 1], f32, tag="ss")
            # ssp = sum_d p^2 (Square+Sqrt share one ACT table set)
            nc.scalar.activation(psq, p_f, AF.Square, accum_out=ssp)
            pnorm = small.tile([P, 1], f32, tag="nrm")
            nc.scalar.activation(pnorm, ssp, AF.Sqrt, bias=tiny_b)
            rp = small.tile([P, 1], f32, tag="rp")
            nc.vector.reciprocal(rp, pnorm)
            rps = small.tile([P, 1], f32, tag="rps")
            nc.vector.tensor_scalar_mul(rps, rp, SCALE_P)
            pn = pnpool.tile([P, D], bf16, tag="pn")
            nc.vector.tensor_scalar_mul(pn, p_f, rps)
            pT_bf = ptpool.tile([P, kt, P], bf16, tag="pT_bf")
            nc.sync.dma_start(pT_bf, pn, transpose=True)
            pT_bfs[mt] = pT_bf
            cast_p(mt - 3)
            # wave-B p load trigger now that pn freed a ppool buf
            nmt = mt + wave_a
            if nmt < mt_n:
                p_f2 = ppool.tile([P, D], f32, tag="p_f")
                nc.sync.dma_start(p_f2, p_d[nmt * P : (nmt + 1) * P, :])
                p_tiles[nmt] = p_f2
        for mt in range(max(0, mt_n - 3), mt_n):
            cast_p(mt)

        for i in range(min(TPIPE, bt_n)):
            trans_x(i)
        cast_x(0)
        norms(0)

        # ---- software-pipelined b-loop ----
        for bt in range(bt_n):
            load_x(bt + XPIPE)
            trans_x(bt + TPIPE)
            cast_x(bt + 1)

            xT8 = xT8s.pop(bt)
            pts = [
                psum.tile([P, 512], f32, tag="ps", name=f"ps_{ci}")[:, :w]
                for ci, (_o, w) in enumerate(chunks)
            ]
            # chunk-major: finish one PSUM bank's accumulation group first so
            # the ACT epilogue starts while later chunks still matmul
            for ci, (coff, w) in enumerate(chunks):
                for j in range(kt // 2):
                    nc.tensor.matmul(
                        pts[ci],
                        xT8[:, 2 * j : 2 * j + 2, :],
                        pT8[:, 2 * j : 2 * j + 2, coff : coff + w],
                        start=(j == 0),
                        stop=(j == kt // 2 - 1),
                        perf_mode=PM.DoubleRow,
                    )
            t_sb = opool.tile([P, mc], bf16, tag="t_sb")
            svec = svecs.pop(bt)
            for ci, (coff, w) in enumerate(chunks):
                # sqrt(s^2/(8||x||) * G' + 2s^2) = s*sqrt(2 - 2*cos)
                nc.scalar.activation(
                    t_sb[:, coff : coff + w], pts[ci], AF.Sqrt,
                    bias=two_s2_b, scale=svec,
                )
            nc.sync.dma_start(o_d[bt * P : (bt + 1) * P, :], t_sb)
            norms(bt + 1)
            x_bfs.pop(bt)

    nc.compile()
    return nc


LAST_RESULT = None


def _run(nc, in_maps, core_ids):
    from concourse import bass_utils

    global LAST_RESULT
    trace = bool(int(os.environ.get("ISOMAX_TRACE", "0")))
    LAST_RESULT = bass_utils.run_bass_kernel_spmd(
        nc, in_maps, core_ids=core_ids, trace=trace
    )
    return LAST_RESULT.results


def kernel(x, prototypes, distance_scale):
    x = np.ascontiguousarray(np.asarray(x, dtype=np.float32))
    p = np.asarray(prototypes, dtype=np.float32)
    s_abs = float(abs(np.asarray(distance_scale).reshape(-1)[0].item()))
    m, d = p.shape
    assert (m, d) == (M_FULL, D) and x.shape == (B, D)

    key = ("fp8", s_abs)
    if key not in _cache:
        _cache[key] = _build(s_abs)
    nc = _cache[key]

    p_pad = np.zeros((N_CORES * MC, D), np.float32)
    p_pad[:m] = p
    in_maps = [
        {"x": x, "p": np.ascontiguousarray(p_pad[i * MC : (i + 1) * MC])}
        for i in range(N_CORES)
    ]
    results = _run(nc, in_maps, list(range(N_CORES)))
    out = np.concatenate(
        [np.asarray(results[i]["o"]) for i in range(N_CORES)], axis=1
    )
    # device emits +|s|*dist; negate during the f32 upcast
    return -(out[:, :m].astype(np.float32))
